# revision 45
# baseline (speedup 1.0000x reference)
"""Causal self-attention (GQA + RoPE) Trainium2 Bass kernel.

Sharding: 8 cores = 2 (batch) x 4 (kv-head groups). Each core computes the
full attention for one batch element and one kv head (with its 4 query
heads), producing a partial output projection (row-split Wproj); the host
sums the 4 kv-group partials per batch element.

Hybrid fp8 scheme: query-chunk 0 (t < 512) runs fully in fp16 (its rows
see few attended positions, so quantization noise does not average out);
chunks 1-3 use fp8e4 DoubleRow matmuls (2x PE rate) for the q/k/v
projections, A*V, the softmax-sum matmul, and the output projection
(fp16 on t-tile 0 only). Scores stay fp16 everywhere. x arrives
pre-transposed from the host, so phase B has no PE transposes.

Scales (static, folded on host): wq8 = Wq*scale*512 (rope tables for
cols>=512 carry /512), wk8 = Wk*64 (tables /64), wv8 = Wv*64 (the
post-transpose copy applies *0.25 so vfin8 = 16*v), wp8 = Wproj*64,
final y copy applies 1/(16*64).

Self-contained: hardcodes B=2, T=2048, E=2048, H=16, HKV=4, D=128.
"""

import sys

for _p in ("/opt/trn_rl_repo", "/root/.axon_site/_ro/trn_rl_repo"):
    if _p not in sys.path:
        sys.path.append(_p)

import math
from contextlib import ExitStack

import numpy as np
import ml_dtypes

import concourse.bacc as bacc
import concourse.tile as tile
import concourse.mybir as mybir
from concourse.bass_utils import run_bass_kernel_spmd

P = 128          # partitions
T = 2048         # sequence length
E = 2048         # embed dim
D = 128          # head dim
GH = 4           # query heads per core (= per kv head)
CH = 512         # t-chunk width (PSUM bank = 512 f32)
NCH = T // CH    # 4 t-chunks
NE = E // P      # 16 contraction chunks over E
NEP = NE // 2    # 8 DoubleRow pairs
NK = T // P      # 16 key tiles
NDIAG = CH // P  # 4 diagonal mask offsets

SQ = 512.0       # wq8 host scale (tables compensate)
SK = 64.0        # wk8 host scale
SVW = 64.0       # wv8 host scale
SV = 16.0        # vfin8 scale (copy applies SV/SVW = 0.25)
SP = 64.0        # wp8 host scale
YS = 1.0 / (SV * SP)

F32 = mybir.dt.float32
F16 = mybir.dt.float16
F8 = mybir.dt.float8e4
E4NP = ml_dtypes.float8_e4m3
DRM = mybir.MatmulPerfMode.DoubleRow
EXPF = mybir.ActivationFunctionType.Exp


def _emit(nc):
    # weights arrive pre-rearranged from the host in exact SBUF layout
    # ([P, ...]) so every DMA is 128 max-length descriptors.
    xt16 = nc.dram_tensor("xt16", [P, NE, CH], F16, kind="ExternalInput")
    xt8 = nc.dram_tensor("xt8", [3, P, NE, CH], F8, kind="ExternalInput")
    wq16 = nc.dram_tensor("wq16", [P, NE, GH * D], F16, kind="ExternalInput")
    wq8 = nc.dram_tensor("wq8", [P, NE, GH * D], F8, kind="ExternalInput")
    wk16 = nc.dram_tensor("wk16", [P, NE, D], F16, kind="ExternalInput")
    wk8 = nc.dram_tensor("wk8", [P, NE, D], F8, kind="ExternalInput")
    wv16 = nc.dram_tensor("wv16", [P, NE, D], F16, kind="ExternalInput")
    wv8 = nc.dram_tensor("wv8", [P, NE, D], F8, kind="ExternalInput")
    cosq = nc.dram_tensor("cosq", [D, T], F16, kind="ExternalInput")
    snq = nc.dram_tensor("snq", [D, T], F16, kind="ExternalInput")
    cosk = nc.dram_tensor("cosk", [D, T], F16, kind="ExternalInput")
    snk = nc.dram_tensor("snk", [D, T], F16, kind="ExternalInput")
    wp16 = nc.dram_tensor("wp16", [P, GH, E], F16, kind="ExternalInput")
    wp8 = nc.dram_tensor("wp8", [P, GH, E], F8, kind="ExternalInput")
    mask = nc.dram_tensor("mask", [P, NDIAG, CH], F16, kind="ExternalInput")
    ident = nc.dram_tensor("ident", [P, P], F16, kind="ExternalInput")
    y = nc.dram_tensor("y", [T, E], F16, kind="ExternalOutput")

    with tile.TileContext(nc) as tc, ExitStack() as ctx:
        # ---- persistent pools (live across phases) ----
        pool_cst = ctx.enter_context(tc.tile_pool(name="cst", bufs=1))
        pool_qfin = ctx.enter_context(tc.tile_pool(name="qfin", bufs=GH))
        pool_kfin = ctx.enter_context(tc.tile_pool(name="kfin", bufs=1))
        pool_vfin = ctx.enter_context(tc.tile_pool(name="vfin", bufs=1))
        pool_outf = ctx.enter_context(tc.tile_pool(name="outf", bufs=1))
        # persistent so ex/rec tiles never alias phase-B scratch (aliasing
        # made the first exp wait on the chunk-3 rope backlog)
        pool_exp = ctx.enter_context(tc.tile_pool(name="expb", bufs=10))
        pool_attw = ctx.enter_context(tc.tile_pool(name="attw", bufs=4))

        ident16 = pool_cst.tile([P, P], F16)
        ones16 = pool_cst.tile([P, P], F16)
        nc.gpsimd.memset(ones16[:], 1.0)
        ones8 = pool_cst.tile([P, 2, P], F8)
        nc.gpsimd.memset(ones8[:], 1.0)
        bias_m2 = pool_cst.tile([P, 1], F32)
        nc.gpsimd.memset(bias_m2[:], -2.0)
        mask_sb = pool_cst.tile([P, NDIAG, CH], F16)

        qfin = [pool_qfin.tile([P, T], F16, tag="qfin", name=f"qfin{h}")
                for h in range(GH)]
        kfin = pool_kfin.tile([P, T], F16)
        vfin16 = pool_vfin.tile([P, NDIAG, P], F16, tag="v16")
        vfin8 = pool_vfin.tile([P, NK, P], F8, tag="v8")
        outf8 = pool_outf.tile([P, GH, T], F8, tag="o8")
        outf16 = pool_outf.tile([P, GH, P], F16, tag="o16")

        def rope_combine(dst_slice, psrc, cos_sl, sn_sl, pool):
            # dst = psrc * cos + rotate_half(psrc) * sn (sn rows 0:64 negated)
            raw = pool.tile([P, CH], F16, tag="rp_raw")
            nc.scalar.copy(raw[:], psrc[:])
            sw = pool.tile([P, CH], F16, tag="rp_sw")
            nc.vector.tensor_copy(sw[0:64, :], raw[64:128, :])
            nc.vector.tensor_copy(sw[64:128, :], raw[0:64, :])
            m1 = pool.tile([P, CH], F16, tag="rp_m1")
            nc.vector.tensor_mul(m1[:], raw[:], cos_sl)
            nc.vector.tensor_mul(sw[:], sw[:], sn_sl)
            nc.vector.tensor_add(dst_slice, m1[:], sw[:])

        # ================= Phase B: projections + RoPE =================
        with (
            tc.tile_pool(name="w16", bufs=1) as pool_w16,
            tc.tile_pool(name="w8", bufs=1) as pool_w8,
            tc.tile_pool(name="tab", bufs=1) as pool_tab,
            tc.tile_pool(name="xt16p", bufs=1) as pool_x16,
            tc.tile_pool(name="xt8p", bufs=3) as pool_x8,
            tc.tile_pool(name="rw", bufs=6) as pool_rw,
            tc.tile_pool(name="vts", bufs=1) as pool_vt,
            tc.tile_pool(name="pstr", bufs=2, space="PSUM") as ps_tr,
            tc.tile_pool(name="pspj", bufs=1, space="PSUM") as ps_pj,
        ):
            # Each dma_start costs ~600ns of serial issue on the Sync
            # engine (descriptors already fan out across all 16 queues),
            # so use FEW, LARGE dmas from host-side SBUF-layout arrays.
            # First pieces kept small so the first matmul starts ASAP.
            xt16_r = pool_x16.tile([P, NE, CH], F16)
            nc.sync.dma_start(xt16_r[:, 0:2, :], xt16.ap()[:, 0:2, :])
            wk16_r = pool_w16.tile([P, NE, D], F16, tag="wk16")
            nc.sync.dma_start(wk16_r[:, 0:8, :], wk16.ap()[:, 0:8, :])
            wv16_r = pool_w16.tile([P, NE, D], F16, tag="wv16")
            nc.sync.dma_start(wv16_r[:, 0:8, :], wv16.ap()[:, 0:8, :])
            nc.sync.dma_start(xt16_r[:, 2:6, :], xt16.ap()[:, 2:6, :])
            nc.sync.dma_start(wk16_r[:, 8:16, :], wk16.ap()[:, 8:16, :])
            nc.sync.dma_start(wv16_r[:, 8:16, :], wv16.ap()[:, 8:16, :])
            nc.sync.dma_start(xt16_r[:, 6:11, :], xt16.ap()[:, 6:11, :])
            nc.sync.dma_start(xt16_r[:, 11:16, :], xt16.ap()[:, 11:16, :])
            wq16_r = pool_w16.tile([P, NE, GH * D], F16, tag="wq16")
            nc.sync.dma_start(wq16_r[:, 0:8, :], wq16.ap()[:, 0:8, :])
            nc.sync.dma_start(wq16_r[:, 8:16, :], wq16.ap()[:, 8:16, :])
            nc.sync.dma_start(ident16[:], ident.ap()[:])

            # fp8 weights BEFORE the xt8 bulk (chunk1 needs them first)
            wk8_r = pool_w8.tile([P, NE, D], F8, tag="wk8")
            nc.sync.dma_start(wk8_r[:], wk8.ap()[:])
            wv8_r = pool_w8.tile([P, NE, D], F8, tag="wv8")
            nc.sync.dma_start(wv8_r[:], wv8.ap()[:])
            cosk_sb = pool_tab.tile([P, T], F16, tag="cosk")
            nc.sync.dma_start(cosk_sb[:], cosk.ap()[:])
            snk_sb = pool_tab.tile([P, T], F16, tag="snk")
            nc.sync.dma_start(snk_sb[:], snk.ap()[:])
            wq8_r = pool_w8.tile([P, NE, GH * D], F8, tag="wq8")
            nc.sync.dma_start(wq8_r[:], wq8.ap()[:])

            xt8_c = {}
            for c in (1, 2, 3):
                xt = pool_x8.tile([P, NE, CH], F8, tag="xt8", name=f"xt8_{c}")
                nc.sync.dma_start(xt[:], xt8.ap()[c - 1])
                xt8_c[c] = xt
                if c == 1:
                    cosq_sb = pool_tab.tile([P, T], F16, tag="cosq")
                    nc.sync.dma_start(cosq_sb[:], cosq.ap()[:])
                    snq_sb = pool_tab.tile([P, T], F16, tag="snq")
                    nc.sync.dma_start(snq_sb[:], snq.ap()[:])
            nc.sync.dma_start(mask_sb[:], mask.ap()[:])

            vt_sb = pool_vt.tile([P, T], F16)

            for c in range(NCH):
                pk = ps_pj.tile([P, CH], F32, tag="pk", bufs=1, name=f"pk{c}")
                pv = ps_pj.tile([P, CH], F32, tag="pv", bufs=1, name=f"pv{c}")
                pqs = [ps_pj.tile([P, CH], F32, tag=f"pq{h}", bufs=1,
                                  name=f"pq{c}_{h}") for h in range(GH)]
                if c == 0:
                    # fp16 chunk; k/v first (their weights land first)
                    for e in range(NE):
                        st = (e == 0)
                        sp = (e == NE - 1)
                        nc.tensor.matmul(pk[:], wk16_r[:, e, :],
                                         xt16_r[:, e, :], start=st, stop=sp)
                        nc.tensor.matmul(pv[:], wv16_r[:, e, :],
                                         xt16_r[:, e, :], start=st, stop=sp)
                    for e in range(NE):
                        for h in range(GH):
                            nc.tensor.matmul(
                                pqs[h][:],
                                wq16_r[:, e, h * D:(h + 1) * D],
                                xt16_r[:, e, :],
                                start=(e == 0), stop=(e == NE - 1),
                            )
                else:
                    xt = xt8_c[c]
                    for ep in range(NEP):
                        st = (ep == 0)
                        sp = (ep == NEP - 1)
                        nc.tensor.matmul(pk[:], wk8_r[:, 2 * ep:2 * ep + 2, :],
                                         xt[:, 2 * ep:2 * ep + 2, :],
                                         perf_mode=DRM, start=st, stop=sp)
                        nc.tensor.matmul(pv[:], wv8_r[:, 2 * ep:2 * ep + 2, :],
                                         xt[:, 2 * ep:2 * ep + 2, :],
                                         perf_mode=DRM, start=st, stop=sp)
                        for h in range(GH):
                            nc.tensor.matmul(
                                pqs[h][:],
                                wq8_r[:, 2 * ep:2 * ep + 2,
                                      h * D:(h + 1) * D],
                                xt[:, 2 * ep:2 * ep + 2, :],
                                perf_mode=DRM, start=st, stop=sp,
                            )
                cs = slice(c * CH, (c + 1) * CH)
                rope_combine(kfin[:, cs], pk, cosk_sb[:, cs], snk_sb[:, cs],
                             pool_rw)
                # v: PSUM -> fp16 (scaled SVW for c>0) -> PE transpose
                # (before q-ropes: AV needs v early, q chunks only later)
                nc.scalar.copy(vt_sb[:, cs], pv[:])
                vtb = ps_tr.tile([P, 4 * P], F16, tag="tr", name=f"vtb{c}")
                for j in range(4):
                    kt = c * 4 + j
                    nc.tensor.matmul(
                        vtb[:, j * P:(j + 1) * P],
                        vt_sb[:, kt * P:(kt + 1) * P],
                        ident16[:],
                        is_transpose=True,
                        start=(j == 0),
                        stop=(j == 3),
                    )
                if c == 0:
                    nc.scalar.copy(vfin16[:, :, :], vtb[:])
                    nc.scalar.mul(vfin8[:, 0:4, :], vtb[:], SV)
                else:
                    nc.scalar.mul(
                        vfin8[:, 4 * c:4 * (c + 1), :], vtb[:], SV / SVW)
                for h in range(GH):
                    rope_combine(qfin[h][:, cs], pqs[h],
                                 cosq_sb[:, cs], snq_sb[:, cs], pool_rw)

        # ================= Phase C: attention =================
        pool_wp = ctx.enter_context(tc.tile_pool(name="wpp", bufs=1))
        wp16_r = pool_wp.tile([P, GH, E], F16, tag="wp16")
        wp8_r = pool_wp.tile([P, GH, E], F8, tag="wp8")
        nc.sync.dma_start(wp16_r[:, 0:2, :], wp16.ap()[:, 0:2, :])
        nc.sync.dma_start(wp16_r[:, 2:4, :], wp16.ap()[:, 2:4, :])
        nc.sync.dma_start(wp8_r[:], wp8.ap()[:])

        with (
            tc.tile_pool(name="scps", bufs=3, space="PSUM") as ps_sc,
            tc.tile_pool(name="avps", bufs=1, space="PSUM") as ps_av,
            tc.tile_pool(name="smps", bufs=1, space="PSUM") as ps_sm,
        ):
            HCH = CH // 2
            for h in range(GH):
                for c in range(NCH):
                    nk = 4 * c + 4
                    npair = nk // 2
                    f8c = (c > 0)
                    exdt = F8 if f8c else F16
                    cs = slice(c * CH, (c + 1) * CH)
                    av = ps_av.tile([P, CH], F32, tag="av", name=f"av{h}_{c}")
                    sm = ps_sm.tile([P, CH], F32, tag="sm", name=f"sm{h}_{c}")
                    exps = {}
                    for kp in range(npair + 1):
                        if kp < npair:
                            # last pair of every chunk covers diag tiles
                            # (j2,j3): queries < 256 are fully masked, so
                            # compute only the upper half-width.
                            trim = (kp == npair - 1)
                            sc = ps_sc.tile([P, 2, CH], F32, tag="sc",
                                            name=f"sc{h}_{c}_{kp}")
                            if trim:
                                qs_n = slice(c * CH + HCH, (c + 1) * CH)
                                for half in (0, 1):
                                    k = 2 * kp + half
                                    nc.tensor.matmul(
                                        sc[:, half, 0:HCH],
                                        kfin[:, k * P:(k + 1) * P],
                                        qfin[h][:, qs_n],
                                        start=True,
                                        stop=True,
                                    )
                                ex = pool_exp.tile([P, 2, HCH], exdt,
                                                   tag="ex2",
                                                   name=f"ex{h}_{c}_{kp}")
                                nc.scalar.activation(ex[:], sc[:, :, 0:HCH],
                                                     EXPF,
                                                     bias=bias_m2[:, 0:1])
                                for half in (0, 1):
                                    nc.vector.tensor_mul(
                                        ex[:, half, :],
                                        ex[:, half, :],
                                        mask_sb[:, 2 + half, HCH:CH],
                                    )
                            else:
                                for half in (0, 1):
                                    k = 2 * kp + half
                                    nc.tensor.matmul(
                                        sc[:, half, :],
                                        kfin[:, k * P:(k + 1) * P],
                                        qfin[h][:, cs],
                                        start=True,
                                        stop=True,
                                    )
                                ex = pool_exp.tile([P, 2, CH], exdt,
                                                   tag="ex",
                                                   name=f"ex{h}_{c}_{kp}")
                                nc.scalar.activation(ex[:], sc[:], EXPF,
                                                     bias=bias_m2[:, 0:1])
                                for half in (0, 1):
                                    m = 2 * kp + half - 4 * c
                                    if m >= 0:
                                        nc.vector.tensor_mul(
                                            ex[:, half, :],
                                            ex[:, half, :],
                                            mask_sb[:, m, :],
                                        )
                            exps[kp] = ex
                        if kp >= 1:
                            kp0 = kp - 1
                            ex = exps.pop(kp0)
                            if f8c:
                                trim0 = (kp0 == npair - 1)
                                avo = av[:, HCH:CH] if trim0 else av[:]
                                smo = sm[:, HCH:CH] if trim0 else sm[:]
                                # group closes at the last FULL-width pair;
                                # the trimmed pair accumulates after (hw:
                                # stop is sim metadata only)
                                stp = (kp0 >= npair - 2)
                                nc.tensor.matmul(
                                    avo,
                                    vfin8[:, 2 * kp0:2 * kp0 + 2, :],
                                    ex[:, :, :],
                                    perf_mode=DRM,
                                    start=(kp0 == 0),
                                    stop=stp,
                                    skip_group_check=trim0,
                                )
                                nc.tensor.matmul(
                                    smo,
                                    ones8[:, :, :],
                                    ex[:, :, :],
                                    perf_mode=DRM,
                                    start=(kp0 == 0),
                                    stop=stp,
                                    skip_group_check=trim0,
                                )
                            else:
                                trim0 = (kp0 == npair - 1)
                                avo = av[:, HCH:CH] if trim0 else av[:]
                                smo = sm[:, HCH:CH] if trim0 else sm[:]
                                for half in (0, 1):
                                    k = 2 * kp0 + half
                                    nc.tensor.matmul(
                                        avo,
                                        vfin16[:, k, :],
                                        ex[:, half, :],
                                        start=(k == 0),
                                        stop=(k >= nk - 3),
                                        skip_group_check=trim0,
                                    )
                                    nc.tensor.matmul(
                                        smo,
                                        ones16[:],
                                        ex[:, half, :],
                                        start=(k == 0),
                                        stop=(k >= nk - 3),
                                        skip_group_check=trim0,
                                    )
                    rec = pool_attw.tile([P, CH], F32, tag="rec")
                    nc.vector.reciprocal_approx_fast(rec[:], sm[:])
                    if c == 0:
                        rec16 = pool_attw.tile([P, CH], F32, tag="rec16")
                        nc.vector.tensor_scalar_mul(rec16[:], rec[:], SV)
                        nc.vector.tensor_mul(
                            outf8[:, h, cs], av[:], rec16[:])
                        nc.vector.tensor_mul(
                            outf16[:, h, :], av[:, 0:P], rec[:, 0:P])
                    else:
                        nc.vector.tensor_mul(
                            outf8[:, h, cs], av[:], rec[:])

        # ================= Phase D: output projection =================
        with (
            tc.tile_pool(name="ystg", bufs=3) as pool_y,
            tc.tile_pool(name="pyps", bufs=4, space="PSUM") as ps_y,
        ):
            for t in range(NK):
                ys = pool_y.tile([P, 4, CH], F16, tag="ys", name=f"ys{t}")
                for eo2 in range(2):
                    py = ps_y.tile([P, 2, CH], F32, tag="py",
                                   name=f"py{t}_{eo2}")
                    for half in (0, 1):
                        eo = 2 * eo2 + half
                        es = slice(eo * CH, (eo + 1) * CH)
                        if t == 0:
                            for j in range(GH):
                                nc.tensor.matmul(
                                    py[:, half, :],
                                    outf16[:, j, :],
                                    wp16_r[:, j, es],
                                    start=(j == 0),
                                    stop=(j == GH - 1),
                                )
                        else:
                            for jp in range(2):
                                nc.tensor.matmul(
                                    py[:, half, :],
                                    outf8[:, 2 * jp:2 * jp + 2,
                                          t * P:(t + 1) * P],
                                    wp8_r[:, 2 * jp:2 * jp + 2, es],
                                    perf_mode=DRM,
                                    start=(jp == 0),
                                    stop=(jp == 1),
                                )
                    # alternate whole-tile copies between scalar and vector:
                    # each engine sees every other py, so neither paces PE.
                    # Last tiles: split halves across BOTH engines so the
                    # final drain chain is short.
                    yso = ys[:, 2 * eo2:2 * eo2 + 2, :]
                    if t >= NK - 2:
                        nc.scalar.mul(yso[:, 0, :], py[:, 0, :], YS)
                        nc.vector.tensor_scalar_mul(
                            yso[:, 1, :], py[:, 1, :], YS)
                    elif t == 0:
                        if eo2 == 0:
                            nc.scalar.copy(yso, py[:])
                        else:
                            nc.vector.tensor_copy(yso, py[:])
                    else:
                        if (2 * t + eo2) % 2 == 0:
                            nc.scalar.mul(yso, py[:], YS)
                        else:
                            nc.vector.tensor_scalar_mul(yso, py[:], YS)
                # one DMA per t-tile (halves the serial sync issue cost);
                # final tile split in two so the last transfer overlaps
                if t == NK - 1:
                    nc.sync.dma_start(
                        y.ap()[t * P:(t + 1) * P, 0:E // 2], ys[:, 0:2, :])
                    nc.sync.dma_start(
                        y.ap()[t * P:(t + 1) * P, E // 2:E], ys[:, 2:4, :])
                else:
                    nc.sync.dma_start(y.ap()[t * P:(t + 1) * P, :], ys[:])

    return nc


_NC = None


def build_nc():
    global _NC
    if _NC is None:
        nc = bacc.Bacc("TRN2", target_bir_lowering=False, debug=False)
        _emit(nc)
        nc.compile()
        _NC = nc
    return _NC


def host_tables(pos):
    """RoPE tables [D, T], mirroring the reference; sn rows 0:half negated."""
    half = D // 2
    inv_freq = (1.0 / np.power(10000.0, np.arange(0, D, 2, dtype=np.float32) / D))
    t = np.arange(pos, pos + T, dtype=np.float32)
    freqs = t[:, None] * inv_freq[None, :]
    freqs = np.repeat(freqs, 2, axis=-1)            # [T, D]
    cos = np.cos(freqs).astype(np.float32).T.copy() # [D, T]
    sin = np.sin(freqs).astype(np.float32).T.copy()
    sn = sin.copy()
    sn[:half] = -sn[:half]
    return cos, sn


def host_masks():
    kk = np.arange(P)[:, None]
    qq = np.arange(CH)[None, :]
    m = np.stack(
        [(kk + 128 * i <= qq) for i in range(NDIAG)], axis=1
    )  # [P, NDIAG, CH]
    return m.astype(np.float16)


def _q8(a, scale=1.0):
    return np.clip(np.asarray(a, np.float32) * scale, -240.0, 240.0).astype(E4NP)


def make_in_maps(x, Wq, Wk, Wv, Wproj, pos):
    x = np.asarray(x, dtype=np.float32)
    Wq = np.asarray(Wq, dtype=np.float32)
    Wk = np.asarray(Wk, dtype=np.float32)
    Wv = np.asarray(Wv, dtype=np.float32)
    Wproj = np.asarray(Wproj, dtype=np.float32)
    scale = np.float32(1.0 / math.sqrt(D))
    cos, sn = host_tables(int(pos))
    # packed per-column scales: cols < CH true, cols >= CH divided
    cosq = cos.copy(); snq = sn.copy()
    cosq[:, CH:] /= SQ; snq[:, CH:] /= SQ
    cosk = cos.copy(); snk = sn.copy()
    cosk[:, CH:] /= SK; snk[:, CH:] /= SK
    maskm = host_masks()
    in_maps = []
    for cidx in range(8):
        b, g = divmod(cidx, 4)
        xT = np.ascontiguousarray(x[b].T)            # [E, T]
        wq_g = Wq[:, g * GH * D:(g + 1) * GH * D]
        wk_g = Wk[:, g * D:(g + 1) * D]
        wv_g = Wv[:, g * D:(g + 1) * D]
        wp_g = Wproj[g * GH * D:(g + 1) * GH * D, :]
        def sb(w):
            # [E, M] -> SBUF layout [P, NE, M] (partition-contiguous rows)
            M = w.shape[1]
            return np.ascontiguousarray(
                w.reshape(NE, P, M).transpose(1, 0, 2))

        def sbp(w):
            # [GH*D, E] -> [P, GH, E]
            return np.ascontiguousarray(
                w.reshape(GH, P, E).transpose(1, 0, 2))

        xt16h = xT[:, :CH].reshape(NE, P, CH).transpose(1, 0, 2)
        xt8h = np.stack([
            _q8(xT[:, c * CH:(c + 1) * CH]).reshape(NE, P, CH)
            .transpose(1, 0, 2) for c in (1, 2, 3)])
        in_maps.append({
            "xt16": np.ascontiguousarray(xt16h).astype(np.float16),
            "xt8": np.ascontiguousarray(xt8h),
            "wq16": sb((wq_g * scale).astype(np.float16)),
            "wq8": sb(_q8(wq_g, scale * SQ)),
            "wk16": sb(wk_g.astype(np.float16)),
            "wk8": sb(_q8(wk_g, SK)),
            "wv16": sb(wv_g.astype(np.float16)),
            "wv8": sb(_q8(wv_g, SVW)),
            "cosq": cosq.astype(np.float16),
            "snq": snq.astype(np.float16),
            "cosk": cosk.astype(np.float16),
            "snk": snk.astype(np.float16),
            "wp16": sbp(wp_g.astype(np.float16)),
            "wp8": sbp(_q8(wp_g, SP)),
            "mask": maskm,
            "ident": np.eye(P, dtype=np.float16),
        })
    return in_maps


def kernel_with_results(x, Wq, Wk, Wv, Wproj, pos, trace=False):
    nc = build_nc()
    in_maps = make_in_maps(x, Wq, Wk, Wv, Wproj, pos)
    res = run_bass_kernel_spmd(nc, in_maps, list(range(8)), trace=trace)
    B = 2
    y = np.zeros((B, T, E), dtype=np.float32)
    for c in range(8):
        b = c // 4
        y[b] += res.results[c]["y"].astype(np.float32)
    return y, res


def kernel(x, Wq, Wk, Wv, Wproj, pos):
    y, _ = kernel_with_results(x, Wq, Wk, Wv, Wproj, pos)
    return y


# revision 46
# speedup vs baseline: 1.0462x; 1.0462x over previous
"""Causal self-attention (GQA + RoPE) Trainium2 Bass kernel.

Sharding: 8 cores = 2 (batch) x 4 (kv-head groups). Each core computes the
full attention for one batch element and one kv head (with its 4 query
heads), producing a partial output projection (row-split Wproj); the host
sums the 4 kv-group partials per batch element.

Hybrid fp8 scheme: query-chunk 0 (t < 512) runs fully in fp16 (its rows
see few attended positions, so quantization noise does not average out);
chunks 1-3 use fp8e4 DoubleRow matmuls (2x PE rate) for the q/k/v
projections, A*V, the softmax-sum matmul, and the output projection
(fp16 on t-tile 0 only). Scores stay fp16 everywhere. x arrives
pre-transposed from the host, so phase B has no PE transposes.

Scales (static, folded on host): wq8 = Wq*scale*512 (rope tables for
cols>=512 carry /512), wk8 = Wk*64 (tables /64), wv8 = Wv*64 (the
post-transpose copy applies *0.25 so vfin8 = 16*v), wp8 = Wproj*64,
final y copy applies 1/(16*64).

Self-contained: hardcodes B=2, T=2048, E=2048, H=16, HKV=4, D=128.
"""

import sys

for _p in ("/opt/trn_rl_repo", "/root/.axon_site/_ro/trn_rl_repo"):
    if _p not in sys.path:
        sys.path.append(_p)

import math
from contextlib import ExitStack

import numpy as np
import ml_dtypes

import concourse.bacc as bacc
import concourse.tile as tile
import concourse.mybir as mybir
from concourse.bass_utils import run_bass_kernel_spmd

P = 128          # partitions
T = 2048         # sequence length
E = 2048         # embed dim
D = 128          # head dim
GH = 4           # query heads per core (= per kv head)
CH = 512         # t-chunk width (PSUM bank = 512 f32)
NCH = T // CH    # 4 t-chunks
NE = E // P      # 16 contraction chunks over E
NEP = NE // 2    # 8 DoubleRow pairs
NK = T // P      # 16 key tiles
NDIAG = CH // P  # 4 diagonal mask offsets

SQ = 512.0       # wq8 host scale (tables compensate)
SK = 64.0        # wk8 host scale
SVW = 64.0       # wv8 host scale
SV = 16.0        # vfin8 scale (copy applies SV/SVW = 0.25)
SP = 64.0        # wp8 host scale
YS = 1.0 / (SV * SP)

F32 = mybir.dt.float32
F16 = mybir.dt.float16
F8 = mybir.dt.float8e4
E4NP = ml_dtypes.float8_e4m3
DRM = mybir.MatmulPerfMode.DoubleRow
EXPF = mybir.ActivationFunctionType.Exp


def _emit(nc):
    # weights arrive pre-rearranged from the host in exact SBUF layout
    # ([P, ...]) so every DMA is 128 max-length descriptors.
    xt16 = nc.dram_tensor("xt16", [P, NE, CH], F16, kind="ExternalInput")
    xt8 = nc.dram_tensor("xt8", [3, P, NE, CH], F8, kind="ExternalInput")
    wq16 = nc.dram_tensor("wq16", [P, NE, GH * D], F16, kind="ExternalInput")
    wq8 = nc.dram_tensor("wq8", [P, NE, GH * D], F8, kind="ExternalInput")
    wk16 = nc.dram_tensor("wk16", [P, NE, D], F16, kind="ExternalInput")
    wk8 = nc.dram_tensor("wk8", [P, NE, D], F8, kind="ExternalInput")
    wv16 = nc.dram_tensor("wv16", [P, NE, D], F16, kind="ExternalInput")
    wv8 = nc.dram_tensor("wv8", [P, NE, D], F8, kind="ExternalInput")
    cosq = nc.dram_tensor("cosq", [D, T], F16, kind="ExternalInput")
    snq = nc.dram_tensor("snq", [D, T], F16, kind="ExternalInput")
    cosk = nc.dram_tensor("cosk", [D, T], F16, kind="ExternalInput")
    snk = nc.dram_tensor("snk", [D, T], F16, kind="ExternalInput")
    wp16 = nc.dram_tensor("wp16", [P, GH, E], F16, kind="ExternalInput")
    wp8 = nc.dram_tensor("wp8", [P, GH, E], F8, kind="ExternalInput")
    mask = nc.dram_tensor("mask", [P, NDIAG, CH], F16, kind="ExternalInput")
    ident = nc.dram_tensor("ident", [P, P], F16, kind="ExternalInput")
    y = nc.dram_tensor("y", [T, E], F16, kind="ExternalOutput")

    with tile.TileContext(nc) as tc, ExitStack() as ctx:
        # ---- persistent pools (live across phases) ----
        pool_cst = ctx.enter_context(tc.tile_pool(name="cst", bufs=1))
        pool_qfin = ctx.enter_context(tc.tile_pool(name="qfin", bufs=GH))
        pool_kfin = ctx.enter_context(tc.tile_pool(name="kfin", bufs=1))
        pool_vfin = ctx.enter_context(tc.tile_pool(name="vfin", bufs=1))
        pool_outf = ctx.enter_context(tc.tile_pool(name="outf", bufs=1))
        # persistent so ex/rec tiles never alias phase-B scratch (aliasing
        # made the first exp wait on the chunk-3 rope backlog)
        pool_exp = ctx.enter_context(tc.tile_pool(name="expb", bufs=8))
        pool_attw = ctx.enter_context(tc.tile_pool(name="attw", bufs=4))

        ident16 = pool_cst.tile([P, P], F16)
        ones16 = pool_cst.tile([P, P], F16)
        nc.gpsimd.memset(ones16[:], 1.0)
        ones8 = pool_cst.tile([P, 2, P], F8)
        nc.gpsimd.memset(ones8[:], 1.0)
        bias_m2 = pool_cst.tile([P, 1], F32)
        nc.gpsimd.memset(bias_m2[:], -2.0)
        mask_sb = pool_cst.tile([P, NDIAG, CH], F16)

        qfin = [pool_qfin.tile([P, T], F16, tag="qfin", name=f"qfin{h}")
                for h in range(GH)]
        kfin = pool_kfin.tile([P, T], F16)
        vfin16 = pool_vfin.tile([P, NDIAG, P], F16, tag="v16")
        vfin8 = pool_vfin.tile([P, NK, P], F8, tag="v8")
        outf8 = pool_outf.tile([P, GH, T], F8, tag="o8")
        outf16 = pool_outf.tile([P, GH, P], F16, tag="o16")

        def rope_combine(dst_slice, psrc, cos_sl, sn_sl, pool):
            # dst = psrc * cos + rotate_half(psrc) * sn (sn rows 0:64 negated)
            raw = pool.tile([P, CH], F16, tag="rp_raw")
            nc.scalar.copy(raw[:], psrc[:])
            sw = pool.tile([P, CH], F16, tag="rp_sw")
            nc.vector.tensor_copy(sw[0:64, :], raw[64:128, :])
            nc.vector.tensor_copy(sw[64:128, :], raw[0:64, :])
            m1 = pool.tile([P, CH], F16, tag="rp_m1")
            nc.vector.tensor_mul(m1[:], raw[:], cos_sl)
            nc.vector.tensor_mul(sw[:], sw[:], sn_sl)
            nc.vector.tensor_add(dst_slice, m1[:], sw[:])

        # ================= Phase B: projections + RoPE =================
        with (
            tc.tile_pool(name="w16", bufs=1) as pool_w16,
            tc.tile_pool(name="w8", bufs=1) as pool_w8,
            tc.tile_pool(name="tab", bufs=1) as pool_tab,
            tc.tile_pool(name="xt16p", bufs=1) as pool_x16,
            tc.tile_pool(name="xt8p", bufs=3) as pool_x8,
            tc.tile_pool(name="rw", bufs=6) as pool_rw,
            tc.tile_pool(name="vts", bufs=1) as pool_vt,
            tc.tile_pool(name="pstr", bufs=2, space="PSUM") as ps_tr,
            tc.tile_pool(name="pspj", bufs=1, space="PSUM") as ps_pj,
        ):
            # Each dma_start costs ~600ns of serial issue on the Sync
            # engine (descriptors already fan out across all 16 queues),
            # so use FEW, LARGE dmas from host-side SBUF-layout arrays.
            # First pieces kept small so the first matmul starts ASAP.
            xt16_r = pool_x16.tile([P, NE, CH], F16)
            nc.sync.dma_start(xt16_r[:, 0:2, :], xt16.ap()[:, 0:2, :])
            wk16_r = pool_w16.tile([P, NE, D], F16, tag="wk16")
            nc.sync.dma_start(wk16_r[:, 0:8, :], wk16.ap()[:, 0:8, :])
            wv16_r = pool_w16.tile([P, NE, D], F16, tag="wv16")
            nc.sync.dma_start(wv16_r[:, 0:8, :], wv16.ap()[:, 0:8, :])
            nc.sync.dma_start(xt16_r[:, 2:6, :], xt16.ap()[:, 2:6, :])
            nc.sync.dma_start(wk16_r[:, 8:16, :], wk16.ap()[:, 8:16, :])
            nc.sync.dma_start(wv16_r[:, 8:16, :], wv16.ap()[:, 8:16, :])
            nc.sync.dma_start(xt16_r[:, 6:11, :], xt16.ap()[:, 6:11, :])
            nc.sync.dma_start(xt16_r[:, 11:16, :], xt16.ap()[:, 11:16, :])
            wq16_r = pool_w16.tile([P, NE, GH * D], F16, tag="wq16")
            nc.sync.dma_start(wq16_r[:, 0:8, :], wq16.ap()[:, 0:8, :])
            nc.sync.dma_start(wq16_r[:, 8:16, :], wq16.ap()[:, 8:16, :])
            nc.sync.dma_start(ident16[:], ident.ap()[:])

            # fp8 weights BEFORE the xt8 bulk (chunk1 needs them first)
            wk8_r = pool_w8.tile([P, NE, D], F8, tag="wk8")
            nc.sync.dma_start(wk8_r[:], wk8.ap()[:])
            wv8_r = pool_w8.tile([P, NE, D], F8, tag="wv8")
            nc.sync.dma_start(wv8_r[:], wv8.ap()[:])
            cosk_sb = pool_tab.tile([P, T], F16, tag="cosk")
            nc.sync.dma_start(cosk_sb[:], cosk.ap()[:])
            snk_sb = pool_tab.tile([P, T], F16, tag="snk")
            nc.sync.dma_start(snk_sb[:], snk.ap()[:])
            wq8_r = pool_w8.tile([P, NE, GH * D], F8, tag="wq8")
            nc.sync.dma_start(wq8_r[:], wq8.ap()[:])

            xt8_c = {}
            for c in (1, 2, 3):
                xt = pool_x8.tile([P, NE, CH], F8, tag="xt8", name=f"xt8_{c}")
                nc.sync.dma_start(xt[:], xt8.ap()[c - 1])
                xt8_c[c] = xt
                if c == 1:
                    cosq_sb = pool_tab.tile([P, T], F16, tag="cosq")
                    nc.sync.dma_start(cosq_sb[:], cosq.ap()[:])
                    snq_sb = pool_tab.tile([P, T], F16, tag="snq")
                    nc.sync.dma_start(snq_sb[:], snq.ap()[:])
            nc.sync.dma_start(mask_sb[:], mask.ap()[:])

            vt_sb = pool_vt.tile([P, T], F16)

            for c in range(NCH):
                pk = ps_pj.tile([P, CH], F32, tag="pk", bufs=1, name=f"pk{c}")
                pv = ps_pj.tile([P, CH], F32, tag="pv", bufs=1, name=f"pv{c}")
                pqs = [ps_pj.tile([P, CH], F32, tag=f"pq{h}", bufs=1,
                                  name=f"pq{c}_{h}") for h in range(GH)]
                if c == 0:
                    # fp16 chunk; k/v first (their weights land first)
                    for e in range(NE):
                        st = (e == 0)
                        sp = (e == NE - 1)
                        nc.tensor.matmul(pk[:], wk16_r[:, e, :],
                                         xt16_r[:, e, :], start=st, stop=sp)
                        nc.tensor.matmul(pv[:], wv16_r[:, e, :],
                                         xt16_r[:, e, :], start=st, stop=sp)
                    for e in range(NE):
                        for h in range(GH):
                            nc.tensor.matmul(
                                pqs[h][:],
                                wq16_r[:, e, h * D:(h + 1) * D],
                                xt16_r[:, e, :],
                                start=(e == 0), stop=(e == NE - 1),
                            )
                else:
                    xt = xt8_c[c]
                    for ep in range(NEP):
                        st = (ep == 0)
                        sp = (ep == NEP - 1)
                        nc.tensor.matmul(pk[:], wk8_r[:, 2 * ep:2 * ep + 2, :],
                                         xt[:, 2 * ep:2 * ep + 2, :],
                                         perf_mode=DRM, start=st, stop=sp)
                        nc.tensor.matmul(pv[:], wv8_r[:, 2 * ep:2 * ep + 2, :],
                                         xt[:, 2 * ep:2 * ep + 2, :],
                                         perf_mode=DRM, start=st, stop=sp)
                        for h in range(GH):
                            nc.tensor.matmul(
                                pqs[h][:],
                                wq8_r[:, 2 * ep:2 * ep + 2,
                                      h * D:(h + 1) * D],
                                xt[:, 2 * ep:2 * ep + 2, :],
                                perf_mode=DRM, start=st, stop=sp,
                            )
                cs = slice(c * CH, (c + 1) * CH)
                rope_combine(kfin[:, cs], pk, cosk_sb[:, cs], snk_sb[:, cs],
                             pool_rw)
                # v: PSUM -> fp16 (scaled SVW for c>0) -> PE transpose
                # (before q-ropes: AV needs v early, q chunks only later)
                nc.scalar.copy(vt_sb[:, cs], pv[:])
                vtb = ps_tr.tile([P, 4 * P], F16, tag="tr", name=f"vtb{c}")
                for j in range(4):
                    kt = c * 4 + j
                    nc.tensor.matmul(
                        vtb[:, j * P:(j + 1) * P],
                        vt_sb[:, kt * P:(kt + 1) * P],
                        ident16[:],
                        is_transpose=True,
                        start=(j == 0),
                        stop=(j == 3),
                    )
                if c == 0:
                    nc.scalar.copy(vfin16[:, :, :], vtb[:])
                    nc.scalar.mul(vfin8[:, 0:4, :], vtb[:], SV)
                else:
                    nc.scalar.mul(
                        vfin8[:, 4 * c:4 * (c + 1), :], vtb[:], SV / SVW)
                for h in range(GH):
                    rope_combine(qfin[h][:, cs], pqs[h],
                                 cosq_sb[:, cs], snq_sb[:, cs], pool_rw)

        # ================= Phase C: attention =================
        pool_wp = ctx.enter_context(tc.tile_pool(name="wpp", bufs=1))
        wp16_r = pool_wp.tile([P, GH, E], F16, tag="wp16")
        wp8_r = pool_wp.tile([P, GH, E], F8, tag="wp8")
        nc.sync.dma_start(wp16_r[:, 0:2, :], wp16.ap()[:, 0:2, :])
        nc.sync.dma_start(wp16_r[:, 2:4, :], wp16.ap()[:, 2:4, :])
        nc.sync.dma_start(wp8_r[:], wp8.ap()[:])

        with (
            tc.tile_pool(name="scps", bufs=3, space="PSUM") as ps_sc,
            tc.tile_pool(name="avps", bufs=1, space="PSUM") as ps_av,
            tc.tile_pool(name="smps", bufs=1, space="PSUM") as ps_sm,
        ):
            HCH = CH // 2
            for h in range(GH):
                for c in range(NCH):
                    nk = 4 * c + 4
                    npair = nk // 2
                    f8c = (c > 0)
                    exdt = F8 if f8c else F16
                    cs = slice(c * CH, (c + 1) * CH)
                    av = ps_av.tile([P, CH], F32, tag="av", name=f"av{h}_{c}")
                    sm = ps_sm.tile([P, CH], F32, tag="sm", name=f"sm{h}_{c}")
                    exps = {}
                    for kp in range(npair + 1):
                        if kp < npair:
                            # last pair of every chunk covers diag tiles
                            # (j2,j3): queries < 256 are fully masked, so
                            # compute only the upper half-width.
                            trim = (kp == npair - 1)
                            sc = ps_sc.tile([P, 2, CH], F32, tag="sc",
                                            name=f"sc{h}_{c}_{kp}")
                            if trim:
                                qs_n = slice(c * CH + HCH, (c + 1) * CH)
                                for half in (0, 1):
                                    k = 2 * kp + half
                                    nc.tensor.matmul(
                                        sc[:, half, 0:HCH],
                                        kfin[:, k * P:(k + 1) * P],
                                        qfin[h][:, qs_n],
                                        start=True,
                                        stop=True,
                                    )
                                ex = pool_exp.tile([P, 2, HCH], exdt,
                                                   tag="ex2",
                                                   name=f"ex{h}_{c}_{kp}")
                                nc.scalar.activation(ex[:], sc[:, :, 0:HCH],
                                                     EXPF,
                                                     bias=bias_m2[:, 0:1])
                                for half in (0, 1):
                                    nc.vector.tensor_mul(
                                        ex[:, half, :],
                                        ex[:, half, :],
                                        mask_sb[:, 2 + half, HCH:CH],
                                    )
                            else:
                                for half in (0, 1):
                                    k = 2 * kp + half
                                    nc.tensor.matmul(
                                        sc[:, half, :],
                                        kfin[:, k * P:(k + 1) * P],
                                        qfin[h][:, cs],
                                        start=True,
                                        stop=True,
                                    )
                                ex = pool_exp.tile([P, 2, CH], exdt,
                                                   tag="ex",
                                                   name=f"ex{h}_{c}_{kp}")
                                nc.scalar.activation(ex[:], sc[:], EXPF,
                                                     bias=bias_m2[:, 0:1])
                                for half in (0, 1):
                                    m = 2 * kp + half - 4 * c
                                    if m >= 0:
                                        nc.vector.tensor_mul(
                                            ex[:, half, :],
                                            ex[:, half, :],
                                            mask_sb[:, m, :],
                                        )
                            exps[kp] = ex
                        if kp >= 1:
                            kp0 = kp - 1
                            ex = exps.pop(kp0)
                            if f8c:
                                trim0 = (kp0 == npair - 1)
                                avo = av[:, HCH:CH] if trim0 else av[:]
                                smo = sm[:, HCH:CH] if trim0 else sm[:]
                                # group closes at the last FULL-width pair;
                                # the trimmed pair accumulates after (hw:
                                # stop is sim metadata only)
                                stp = (kp0 >= npair - 2)
                                nc.tensor.matmul(
                                    avo,
                                    vfin8[:, 2 * kp0:2 * kp0 + 2, :],
                                    ex[:, :, :],
                                    perf_mode=DRM,
                                    start=(kp0 == 0),
                                    stop=stp,
                                    skip_group_check=trim0,
                                )
                                nc.tensor.matmul(
                                    smo,
                                    ones8[:, :, :],
                                    ex[:, :, :],
                                    perf_mode=DRM,
                                    start=(kp0 == 0),
                                    stop=stp,
                                    skip_group_check=trim0,
                                )
                            else:
                                trim0 = (kp0 == npair - 1)
                                avo = av[:, HCH:CH] if trim0 else av[:]
                                smo = sm[:, HCH:CH] if trim0 else sm[:]
                                for half in (0, 1):
                                    k = 2 * kp0 + half
                                    nc.tensor.matmul(
                                        avo,
                                        vfin16[:, k, :],
                                        ex[:, half, :],
                                        start=(k == 0),
                                        stop=(k >= nk - 3),
                                        skip_group_check=trim0,
                                    )
                                    nc.tensor.matmul(
                                        smo,
                                        ones16[:],
                                        ex[:, half, :],
                                        start=(k == 0),
                                        stop=(k >= nk - 3),
                                        skip_group_check=trim0,
                                    )
                    rec = pool_attw.tile([P, CH], F32, tag="rec")
                    nc.vector.reciprocal_approx_fast(rec[:], sm[:])
                    if c == 0:
                        rec16 = pool_attw.tile([P, CH], F32, tag="rec16")
                        nc.vector.tensor_scalar_mul(rec16[:], rec[:], SV)
                        nc.vector.tensor_mul(
                            outf8[:, h, cs], av[:], rec16[:])
                        nc.vector.tensor_mul(
                            outf16[:, h, :], av[:, 0:P], rec[:, 0:P])
                    else:
                        nc.vector.tensor_mul(
                            outf8[:, h, cs], av[:], rec[:])

        # ================= Phase D: output projection =================
        with (
            tc.tile_pool(name="ystg", bufs=3) as pool_y,
            tc.tile_pool(name="pyps", bufs=4, space="PSUM") as ps_y,
        ):
            for t in range(NK):
                ys = pool_y.tile([P, 4, CH], F16, tag="ys", name=f"ys{t}")
                for eo2 in range(2):
                    py = ps_y.tile([P, 2, CH], F32, tag="py",
                                   name=f"py{t}_{eo2}")
                    for half in (0, 1):
                        eo = 2 * eo2 + half
                        es = slice(eo * CH, (eo + 1) * CH)
                        if t == 0:
                            for j in range(GH):
                                nc.tensor.matmul(
                                    py[:, half, :],
                                    outf16[:, j, :],
                                    wp16_r[:, j, es],
                                    start=(j == 0),
                                    stop=(j == GH - 1),
                                )
                        else:
                            for jp in range(2):
                                nc.tensor.matmul(
                                    py[:, half, :],
                                    outf8[:, 2 * jp:2 * jp + 2,
                                          t * P:(t + 1) * P],
                                    wp8_r[:, 2 * jp:2 * jp + 2, es],
                                    perf_mode=DRM,
                                    start=(jp == 0),
                                    stop=(jp == 1),
                                )
                    # alternate whole-tile copies between scalar and vector:
                    # each engine sees every other py, so neither paces PE.
                    # Last tiles: split halves across BOTH engines so the
                    # final drain chain is short.
                    yso = ys[:, 2 * eo2:2 * eo2 + 2, :]
                    if t >= NK - 2:
                        nc.scalar.mul(yso[:, 0, :], py[:, 0, :], YS)
                        nc.vector.tensor_scalar_mul(
                            yso[:, 1, :], py[:, 1, :], YS)
                    elif t == 0:
                        if eo2 == 0:
                            nc.scalar.copy(yso, py[:])
                        else:
                            nc.vector.tensor_copy(yso, py[:])
                    else:
                        if (2 * t + eo2) % 2 == 0:
                            nc.scalar.mul(yso, py[:], YS)
                        else:
                            nc.vector.tensor_scalar_mul(yso, py[:], YS)
                # one DMA per t-tile (halves the serial sync issue cost);
                # final tile split in two so the last transfer overlaps
                if t == NK - 1:
                    nc.sync.dma_start(
                        y.ap()[t * P:(t + 1) * P, 0:E // 2], ys[:, 0:2, :])
                    nc.sync.dma_start(
                        y.ap()[t * P:(t + 1) * P, E // 2:E], ys[:, 2:4, :])
                else:
                    nc.sync.dma_start(y.ap()[t * P:(t + 1) * P, :], ys[:])

    return nc


_NC = None


def build_nc():
    global _NC
    if _NC is None:
        nc = bacc.Bacc("TRN2", target_bir_lowering=False, debug=False)
        _emit(nc)
        nc.compile()
        _NC = nc
    return _NC


def host_tables(pos):
    """RoPE tables [D, T], mirroring the reference; sn rows 0:half negated."""
    half = D // 2
    inv_freq = (1.0 / np.power(10000.0, np.arange(0, D, 2, dtype=np.float32) / D))
    t = np.arange(pos, pos + T, dtype=np.float32)
    freqs = t[:, None] * inv_freq[None, :]
    freqs = np.repeat(freqs, 2, axis=-1)            # [T, D]
    cos = np.cos(freqs).astype(np.float32).T.copy() # [D, T]
    sin = np.sin(freqs).astype(np.float32).T.copy()
    sn = sin.copy()
    sn[:half] = -sn[:half]
    return cos, sn


def host_masks():
    kk = np.arange(P)[:, None]
    qq = np.arange(CH)[None, :]
    m = np.stack(
        [(kk + 128 * i <= qq) for i in range(NDIAG)], axis=1
    )  # [P, NDIAG, CH]
    return m.astype(np.float16)


def _q8(a, scale=1.0):
    return np.clip(np.asarray(a, np.float32) * scale, -240.0, 240.0).astype(E4NP)


def make_in_maps(x, Wq, Wk, Wv, Wproj, pos):
    x = np.asarray(x, dtype=np.float32)
    Wq = np.asarray(Wq, dtype=np.float32)
    Wk = np.asarray(Wk, dtype=np.float32)
    Wv = np.asarray(Wv, dtype=np.float32)
    Wproj = np.asarray(Wproj, dtype=np.float32)
    scale = np.float32(1.0 / math.sqrt(D))
    cos, sn = host_tables(int(pos))
    # packed per-column scales: cols < CH true, cols >= CH divided
    cosq = cos.copy(); snq = sn.copy()
    cosq[:, CH:] /= SQ; snq[:, CH:] /= SQ
    cosk = cos.copy(); snk = sn.copy()
    cosk[:, CH:] /= SK; snk[:, CH:] /= SK
    maskm = host_masks()
    in_maps = []
    for cidx in range(8):
        b, g = divmod(cidx, 4)
        xT = np.ascontiguousarray(x[b].T)            # [E, T]
        wq_g = Wq[:, g * GH * D:(g + 1) * GH * D]
        wk_g = Wk[:, g * D:(g + 1) * D]
        wv_g = Wv[:, g * D:(g + 1) * D]
        wp_g = Wproj[g * GH * D:(g + 1) * GH * D, :]
        def sb(w):
            # [E, M] -> SBUF layout [P, NE, M] (partition-contiguous rows)
            M = w.shape[1]
            return np.ascontiguousarray(
                w.reshape(NE, P, M).transpose(1, 0, 2))

        def sbp(w):
            # [GH*D, E] -> [P, GH, E]
            return np.ascontiguousarray(
                w.reshape(GH, P, E).transpose(1, 0, 2))

        xt16h = xT[:, :CH].reshape(NE, P, CH).transpose(1, 0, 2)
        xt8h = np.stack([
            _q8(xT[:, c * CH:(c + 1) * CH]).reshape(NE, P, CH)
            .transpose(1, 0, 2) for c in (1, 2, 3)])
        in_maps.append({
            "xt16": np.ascontiguousarray(xt16h).astype(np.float16),
            "xt8": np.ascontiguousarray(xt8h),
            "wq16": sb((wq_g * scale).astype(np.float16)),
            "wq8": sb(_q8(wq_g, scale * SQ)),
            "wk16": sb(wk_g.astype(np.float16)),
            "wk8": sb(_q8(wk_g, SK)),
            "wv16": sb(wv_g.astype(np.float16)),
            "wv8": sb(_q8(wv_g, SVW)),
            "cosq": cosq.astype(np.float16),
            "snq": snq.astype(np.float16),
            "cosk": cosk.astype(np.float16),
            "snk": snk.astype(np.float16),
            "wp16": sbp(wp_g.astype(np.float16)),
            "wp8": sbp(_q8(wp_g, SP)),
            "mask": maskm,
            "ident": np.eye(P, dtype=np.float16),
        })
    return in_maps


def kernel_with_results(x, Wq, Wk, Wv, Wproj, pos, trace=False):
    nc = build_nc()
    in_maps = make_in_maps(x, Wq, Wk, Wv, Wproj, pos)
    res = run_bass_kernel_spmd(nc, in_maps, list(range(8)), trace=trace)
    B = 2
    y = np.zeros((B, T, E), dtype=np.float32)
    for c in range(8):
        b = c // 4
        y[b] += res.results[c]["y"].astype(np.float32)
    return y, res


def kernel(x, Wq, Wk, Wv, Wproj, pos):
    y, _ = kernel_with_results(x, Wq, Wk, Wv, Wproj, pos)
    return y
